# revision 6
# baseline (speedup 1.0000x reference)
"""Trainium2 Bass kernel for nn_Actor_Soft_Attention (gnn_message_passing).

Reference computation (B=65536, IN=128, HID=256, OUT=8):
    agents  = [[x0,x1],[x0,x2]]                 # [B,2,2*IN]
    h_ij    = relu(agents @ W1.T + b1)          # [B,2,HID]
    e_ij    = relu(agents @ W2.T + b2)
    a_ij    = softmax(e_ij, axis=1)             # over the 2 neighbors
    h_i     = sum(a_ij * h_ij, axis=1)          # [B,HID]
    xxx     = relu([h_i, x0,x1,x2] @ W3.T + b3) # [B,HID]
    out     = tanh(xxx @ W4.T + b4)             # [B,OUT]

Sharding: pure data parallel over the batch dim, 8192 rows per core on 8
NeuronCores. Weights replicated. The host pre-transposes state to a
feature-major [3, 128, 8192] bf16 layout per core so the device needs no
on-chip transposes; 2-way softmax is computed as sigmoid(e1-e2).
"""

import os
import numpy as np
import ml_dtypes

B, IN, HID, OUT = 65536, 128, 256, 8
NCORES = 8
BS = B // NCORES          # rows per core
NT = 512                  # batch columns per tile
T = BS // NT              # tiles per core


def build_nc():
    from concourse import bacc, mybir
    from concourse import tile as tile_mod
    import concourse.bass as bass

    dt = mybir.dt
    f32, bf16 = dt.float32, dt.bfloat16
    AF = mybir.ActivationFunctionType
    ALU = mybir.AluOpType

    nc = bacc.Bacc("TRN2", target_bir_lowering=False, debug=False)

    xt = nc.declare_dram_parameter("xt", [3, 128, BS], bf16, isOutput=False)
    w1 = nc.declare_dram_parameter("w1t", [2, 128, 256], bf16, isOutput=False)
    w2 = nc.declare_dram_parameter("w2t", [2, 128, 256], bf16, isOutput=False)
    w3 = nc.declare_dram_parameter("w3t", [5, 128, 256], bf16, isOutput=False)
    w4 = nc.declare_dram_parameter("w4t", [2, 128, OUT], bf16, isOutput=False)
    b1 = nc.declare_dram_parameter("b1", [2, 128, 1], f32, isOutput=False)
    b2 = nc.declare_dram_parameter("b2", [2, 128, 1], f32, isOutput=False)
    b3 = nc.declare_dram_parameter("b3", [2, 128, 1], f32, isOutput=False)
    b4 = nc.declare_dram_parameter("b4", [OUT, 1], f32, isOutput=False)
    out_d = nc.declare_dram_parameter("out", [OUT, BS], f32, isOutput=True)

    with tile_mod.TileContext(nc) as tc:
        with (
            tc.tile_pool(name="const", bufs=1) as cpool,
            tc.tile_pool(name="xin", bufs=3) as xpool,
            tc.tile_pool(name="act", bufs=2) as apool,
            tc.tile_pool(name="psum", bufs=1, space="PSUM") as ppool,
        ):
            # ---- replicated weights/biases, loaded once ----
            w1_sb = cpool.tile([128, 512], bf16)
            w2_sb = cpool.tile([128, 512], bf16)
            w3_sb = cpool.tile([128, 1280], bf16)
            w4_sb = cpool.tile([128, 2 * OUT], bf16)
            for k in range(2):
                nc.sync.dma_start(out=w1_sb[:, k * 256 : (k + 1) * 256], in_=w1[k])
                nc.sync.dma_start(out=w2_sb[:, k * 256 : (k + 1) * 256], in_=w2[k])
                nc.sync.dma_start(out=w4_sb[:, k * OUT : (k + 1) * OUT], in_=w4[k])
            for k in range(5):
                nc.sync.dma_start(out=w3_sb[:, k * 256 : (k + 1) * 256], in_=w3[k])
            b1_sb = cpool.tile([128, 2], f32)
            b2_sb = cpool.tile([128, 2], f32)
            b3_sb = cpool.tile([128, 2], f32)
            b4_sb = cpool.tile([OUT, 1], f32)
            for k in range(2):
                nc.sync.dma_start(out=b1_sb[:, k : k + 1], in_=b1[k])
                nc.sync.dma_start(out=b2_sb[:, k : k + 1], in_=b2[k])
                nc.sync.dma_start(out=b3_sb[:, k : k + 1], in_=b3[k])
            nc.sync.dma_start(out=b4_sb[:], in_=b4[:])

            def w1c(k, m):
                return w1_sb[:, k * 256 + m * 128 : k * 256 + (m + 1) * 128]

            def w2c(k, m):
                return w2_sb[:, k * 256 + m * 128 : k * 256 + (m + 1) * 128]

            def w3c(k, m):
                return w3_sb[:, k * 256 + m * 128 : k * 256 + (m + 1) * 128]

            def w4c(k):
                return w4_sb[:, k * OUT : (k + 1) * OUT]

            for t in range(T):
                ts = slice(t * NT, (t + 1) * NT)

                x_sb = xpool.tile([128, 3 * NT], bf16, tag="x")
                for j in range(3):
                    nc.sync.dma_start(
                        out=x_sb[:, j * NT : (j + 1) * NT], in_=xt[j, :, ts]
                    )

                def X(j):
                    return x_sb[:, j * NT : (j + 1) * NT]

                # ---- e_ij = relu(W2 @ agents + b2), r = [e1 | e2] per m ----
                rr = []
                for m in (0, 1):
                    r = apool.tile([128, 2 * NT], bf16, tag="r")
                    for n in (0, 1):
                        ps = ppool.tile([128, NT], f32, tag="psE", bufs=3)
                        nc.tensor.matmul(ps[:], w2c(0, m), X(0), start=True, stop=False)
                        nc.tensor.matmul(ps[:], w2c(1, m), X(1 + n), start=False, stop=True)
                        nc.scalar.activation(
                            r[:, n * NT : (n + 1) * NT], ps[:], AF.Relu,
                            bias=b2_sb[:, m : m + 1],
                        )
                    rr.append(r)

                # d = e1 - e2 (both m chunks packed); a1 = sigmoid(d)
                dvec = apool.tile([128, 2 * NT], bf16, tag="d")
                for m in (0, 1):
                    nc.vector.tensor_sub(
                        dvec[:, m * NT : (m + 1) * NT],
                        rr[m][:, 0:NT], rr[m][:, NT : 2 * NT],
                    )
                avec = apool.tile([128, 2 * NT], bf16, tag="a")
                nc.scalar.activation(avec[:], dvec[:], AF.Sigmoid)

                # ---- h_ij = relu(W1 @ agents + b1), hh = [h1 | h2] per m ----
                hh = []
                for m in (0, 1):
                    h = apool.tile([128, 2 * NT], bf16, tag="h")
                    for n in (0, 1):
                        ps = ppool.tile([128, NT], f32, tag="psH", bufs=2)
                        nc.tensor.matmul(ps[:], w1c(0, m), X(0), start=True, stop=False)
                        nc.tensor.matmul(ps[:], w1c(1, m), X(1 + n), start=False, stop=True)
                        nc.vector.tensor_scalar(
                            h[:, n * NT : (n + 1) * NT], ps[:],
                            b1_sb[:, m : m + 1], 0.0, ALU.add, ALU.max,
                        )
                    hh.append(h)

                # ---- h_i = a1*(h1-h2) + h2 per m ----
                hi = []
                for m in (0, 1):
                    t_sb = apool.tile([128, NT], bf16, tag="t")
                    nc.vector.tensor_sub(t_sb[:], hh[m][:, 0:NT], hh[m][:, NT : 2 * NT])
                    u_sb = apool.tile([128, NT], bf16, tag="u")
                    nc.vector.tensor_mul(u_sb[:], avec[:, m * NT : (m + 1) * NT], t_sb[:])
                    hi_m = apool.tile([128, NT], bf16, tag="hi")
                    nc.vector.tensor_add(hi_m[:], u_sb[:], hh[m][:, NT : 2 * NT])
                    hi.append(hi_m)

                # ---- xxx = relu(W3 @ [h_i; x0; x1; x2] + b3) ----
                xxx = []
                for m in (0, 1):
                    ps = ppool.tile([128, NT], f32, tag="psX", bufs=2)
                    nc.tensor.matmul(ps[:], w3c(0, m), hi[0][:], start=True, stop=False)
                    nc.tensor.matmul(ps[:], w3c(1, m), hi[1][:], start=False, stop=False)
                    for j in range(3):
                        nc.tensor.matmul(
                            ps[:], w3c(2 + j, m), X(j), start=False, stop=(j == 2)
                        )
                    xm = apool.tile([128, NT], bf16, tag="xx")
                    nc.scalar.activation(
                        xm[:], ps[:], AF.Relu, bias=b3_sb[:, m : m + 1]
                    )
                    xxx.append(xm)

                # ---- out = tanh(W4 @ xxx + b4) ----
                psO = ppool.tile([OUT, NT], f32, tag="psO", bufs=1)
                nc.tensor.matmul(psO[:], w4c(0), xxx[0][:], start=True, stop=False)
                nc.tensor.matmul(psO[:], w4c(1), xxx[1][:], start=False, stop=True)
                o_sb = apool.tile([OUT, NT], f32, tag="o")
                nc.scalar.activation(o_sb[:], psO[:], AF.Tanh, bias=b4_sb[:])
                nc.sync.dma_start(out=out_d[:, ts], in_=o_sb[:])

    nc.compile()
    return nc


_NC_CACHE = None


def _get_nc():
    global _NC_CACHE
    if _NC_CACHE is None:
        _NC_CACHE = build_nc()
    return _NC_CACHE


def _prep_in_maps(state, W1, b1, W2, b2, W3, b3, W4, b4):
    bf16 = ml_dtypes.bfloat16
    state = np.asarray(state, np.float32).reshape(B, 3, IN)

    def t_chunks(W, kchunks):
        # W [out, in] -> W.T [in, out] -> [kchunks, 128, out] bf16
        Wt = np.ascontiguousarray(np.asarray(W, np.float32).T)
        return Wt.reshape(kchunks, 128, Wt.shape[1]).astype(bf16)

    w1t = t_chunks(W1, 2)
    w2t = t_chunks(W2, 2)
    w3t = t_chunks(W3, 5)
    w4t = t_chunks(W4, 2)
    b1r = np.asarray(b1, np.float32).reshape(2, 128, 1)
    b2r = np.asarray(b2, np.float32).reshape(2, 128, 1)
    b3r = np.asarray(b3, np.float32).reshape(2, 128, 1)
    b4r = np.asarray(b4, np.float32).reshape(OUT, 1)

    in_maps = []
    for c in range(NCORES):
        shard = state[c * BS : (c + 1) * BS]              # [BS, 3, 128]
        xtc = np.ascontiguousarray(shard.transpose(1, 2, 0)).astype(bf16)
        in_maps.append({
            "xt": xtc, "w1t": w1t, "w2t": w2t, "w3t": w3t, "w4t": w4t,
            "b1": b1r, "b2": b2r, "b3": b3r, "b4": b4r,
        })
    return in_maps


def _ensure_ntff_hook():
    """Register the axon NTFF profile hook if the image's antenv lacks it."""
    import sys, types
    try:
        from antenv.axon_hooks import get_axon_ntff_profile_hook  # noqa: F401
        return
    except ImportError:
        pass
    from trn_agent_boot.trn_boot import _ntff_profile_via_ctypes
    hook = _ntff_profile_via_ctypes("/opt/axon/libaxon_pjrt.so")
    mod = types.ModuleType("antenv.axon_hooks")
    holder = {"hook": hook}
    mod.get_axon_ntff_profile_hook = lambda: holder["hook"]
    mod.set_axon_ntff_profile_hook = lambda h: holder.__setitem__("hook", h)
    sys.modules["antenv.axon_hooks"] = mod
    import antenv
    antenv.axon_hooks = mod


def run(inputs, trace=False):
    """Compile (cached), run on 8 cores, return (full_output, exec_time_ns)."""
    from concourse import bass_utils
    from concourse.bass_utils import run_bass_kernel_spmd

    if trace:
        _ensure_ntff_hook()
        bass_utils.upload_artifacts = lambda tmpdir: tmpdir  # no S3 here

    nc = _get_nc()
    in_maps = _prep_in_maps(**inputs)
    res = run_bass_kernel_spmd(nc, in_maps, core_ids=list(range(NCORES)), trace=trace)
    out = np.concatenate([res.results[c]["out"].T for c in range(NCORES)], axis=0)
    return np.ascontiguousarray(out, dtype=np.float32), res.exec_time_ns


def kernel(**inputs) -> np.ndarray:
    out, _ = run(inputs, trace=bool(os.environ.get("KERNEL_TRACE")))
    return out
